# revision 19
# baseline (speedup 1.0000x reference)
"""Multi-head attention (B=4, S=2048, D=1024, H=16, Hd=64) on 8 trn2 cores.

Sharding: core c = (batch b = c // 2, head-group hg = c % 2). Each core
computes attention for 8 heads of one batch and the corresponding slice of
the output projection; host sums the two partial outputs per batch.

v2: single merged pipeline. The attention phase is ScalarE-bound (exp);
all projection / O-proj matmuls are woven between attention iterations as
filler units so the PE never idles while exp runs. ScalarE does ONLY exp;
every PSUM->SBUF move is on the Vector engine; reciprocal uses the fast
approximation.

Per-core layout (matmuls bf16, fp32 PSUM):
  xt    = x[b].T                  [D=1024, S=2048]
  qT/kT = (Wslice.T @ xt)         [128, 4, 2048]  d-major pair-tiles
  v     = xt.T @ Wv_slice         [128, 16, 8, 65]  (65th col = ones -> denom)
  attn chunk (c, p): i-loop over 16 k-tiles:
     scoresT[k, q] pair tile = kT-half.T @ qT-half (row-split concurrency)
     exp (ScalarE, scale=1/8) -> et bf16
     outT[d, q] += v-block.T @ et ; row 64 accumulates denom
  normalize (deferred 1 chunk): 1/denom via reciprocal_approx_fast,
     broadcast with K=1 ones-matmul, DVE multiply
  y = aoT.T-blocks @ Wo_slice   [2048, 1024] fp32 partial, per c-group
"""

from collections import deque

import numpy as np
import ml_dtypes

S = 2048
D = 1024
HG_D = 512          # head dims per core (8 heads x 64)
NH = 8              # heads per core
KT = S // 128       # 16 k-tiles
DT = D // 128       # 8 contraction tiles for QKV
ST = S // 128       # 16 s-tiles
OT = HG_D // 128    # 4 pair-tiles / contraction tiles for O-proj
N_CORES = 8

BF16 = ml_dtypes.bfloat16

_CACHED_NC = {}


def _build_nc(with_bq=False, with_bk=False, with_bv=False, with_bo=False):
    import concourse.bass as bass  # noqa: F401
    import concourse.mybir as mybir
    import concourse.tile as tile
    from concourse import bacc

    f32 = mybir.dt.float32
    bf16 = mybir.dt.bfloat16
    Exp = mybir.ActivationFunctionType.Exp

    nc = bacc.Bacc("TRN2", target_bir_lowering=False, debug=False,
                   num_devices=N_CORES)

    xt_d = nc.dram_tensor("xt", [D, S], bf16, kind="ExternalInput")
    wq_d = nc.dram_tensor("wq", [D, HG_D], bf16, kind="ExternalInput")
    wk_d = nc.dram_tensor("wk", [D, HG_D], bf16, kind="ExternalInput")
    wv_d = nc.dram_tensor("wv", [D, HG_D], bf16, kind="ExternalInput")
    wo_d = nc.dram_tensor("wo", [HG_D, D], bf16, kind="ExternalInput")
    bq_d = nc.dram_tensor("bqt", [128, OT], f32, kind="ExternalInput")
    bk_d = nc.dram_tensor("bkt", [128, OT], f32, kind="ExternalInput")
    bv_d = nc.dram_tensor("bvr", [1, HG_D], bf16, kind="ExternalInput")
    bo_d = nc.dram_tensor("bor", [1, D], bf16, kind="ExternalInput")
    y_d = nc.dram_tensor("y", [S, D], f32, kind="ExternalOutput")

    with tile.TileContext(nc) as tc:
        with (
            tc.tile_pool(name="cpool", bufs=1) as cpool,
            tc.tile_pool(name="wpool", bufs=2) as wpool,
            tc.tile_pool(name="pspool", bufs=2, space="PSUM") as pspool,
            tc.tile_pool(name="popool", bufs=2, space="PSUM") as popool,
            tc.tile_pool(name="paux", bufs=2, space="PSUM") as paux,
        ):
            # ---- persistent SBUF tiles ----
            xt_sb = cpool.tile([128, DT, S], bf16, name="xt_sb")
            wq_sb = cpool.tile([128, DT, HG_D], bf16, name="wq_sb")
            wk_sb = cpool.tile([128, DT, HG_D], bf16, name="wk_sb")
            wv_sb = cpool.tile([128, DT, HG_D], bf16, name="wv_sb")
            wo_sb = cpool.tile([128, OT, D], bf16, name="wo_sb")
            bq_sb = cpool.tile([128, OT], f32, name="bq_sb")
            bk_sb = cpool.tile([128, OT], f32, name="bk_sb")
            bvr_sb = cpool.tile([1, HG_D], bf16, name="bvr_sb")
            bor_sb = cpool.tile([1, D], bf16, name="bor_sb")
            ones_t = cpool.tile([128, 128], bf16, name="ones_t")
            qT_sb = cpool.tile([128, OT, S], bf16, name="qT_sb")
            kT_sb = cpool.tile([128, OT, S], bf16, name="kT_sb")
            # v with a trailing ones column per head: attnv lhsT [128, 65]
            # whose 65th output row accumulates the softmax denominator.
            v_sb = cpool.tile([128, ST, NH, 65], bf16, name="v_sb")
            aoT_sb = cpool.tile([128, OT, S], bf16, name="aoT_sb")
            den_sb = cpool.tile([97, 512], f32, name="den_sb")

            # ---- loads: xt + wk + wq first (lead-in needs them) ----
            for k in range(DT):
                nc.sync.dma_start(out=xt_sb[:, k, :], in_=xt_d[k * 128:(k + 1) * 128, :])
                nc.sync.dma_start(out=wk_sb[:, k, :], in_=wk_d[k * 128:(k + 1) * 128, :])
                nc.sync.dma_start(out=wq_sb[:, k, :], in_=wq_d[k * 128:(k + 1) * 128, :])
            for k in range(DT):
                nc.sync.dma_start(out=wv_sb[:, k, :], in_=wv_d[k * 128:(k + 1) * 128, :])
            for k in range(OT):
                nc.sync.dma_start(out=wo_sb[:, k, :], in_=wo_d[k * 128:(k + 1) * 128, :])
            if with_bq:
                nc.sync.dma_start(out=bq_sb[:], in_=bq_d[:])
            if with_bk:
                nc.sync.dma_start(out=bk_sb[:], in_=bk_d[:])
            if with_bv:
                nc.sync.dma_start(out=bvr_sb[:], in_=bv_d[:])
            if with_bo:
                nc.sync.dma_start(out=bor_sb[:], in_=bo_d[:])
            nc.gpsimd.memset(ones_t[:], 1.0)
            nc.vector.memset(v_sb[:], 1.0)
            nc.vector.memset(den_sb[:], 1.0)

            # ---- projection emitters (as 2-MM units for weaving) ----
            def emit_qk_quarter(w_sb, b_sb, out_sb, p, jc, quarter, pq, withb):
                for k in range(2 * quarter, 2 * quarter + 2):
                    nc.tensor.matmul(
                        pq[:],
                        w_sb[:, k, p * 128:(p + 1) * 128],
                        xt_sb[:, k, jc * 512:(jc + 1) * 512],
                        start=(k == 0), stop=(k == DT - 1),
                    )
                if quarter == 3:
                    dst = out_sb[:, p, jc * 512:(jc + 1) * 512]
                    if withb:
                        nc.scalar.add(dst, pq[:], b_sb[:, p:p + 1])
                    else:
                        nc.vector.tensor_copy(dst, pq[:])

            def qk_group_units(w_sb, b_sb, out_sb, p, jc, withb):
                pq = paux.tile([128, 512], f32, tag="aux", name="pq")
                for quarter in range(4):
                    yield (lambda q=quarter: emit_qk_quarter(
                        w_sb, b_sb, out_sb, p, jc, q, pq, withb))

            def emit_v_quarter(st, quarter, pv):
                for k in range(2 * quarter, 2 * quarter + 2):
                    nc.tensor.matmul(
                        pv[:],
                        xt_sb[:, k, st * 128:(st + 1) * 128],
                        wv_sb[:, k, :],
                        start=(k == 0),
                        stop=(not with_bv and k == DT - 1),
                    )
                if quarter == 3:
                    if with_bv:
                        nc.tensor.matmul(pv[:], ones_t[0:1, 0:128],
                                         bvr_sb[0:1, :], start=False, stop=True)
                    nc.vector.tensor_copy(
                        v_sb[:, st, :, 0:64],
                        pv.rearrange("p (h c) -> p h c", c=64))

            def v_group_units(st):
                pv = paux.tile([128, 512], f32, tag="aux", name="pv")
                for quarter in range(4):
                    yield (lambda q=quarter: emit_v_quarter(st, q, pv))

            def emit_oproj_half(st, l, half, py, yt):
                ks = range(2) if half == 0 else range(2, 4)
                for kt in ks:
                    nc.tensor.matmul(
                        py[:],
                        aoT_sb[:, kt, st * 128:(st + 1) * 128],
                        wo_sb[:, kt, l * 512:(l + 1) * 512],
                        start=(kt == 0),
                        stop=(not with_bo and kt == OT - 1),
                    )
                if half == 1:
                    if with_bo:
                        nc.tensor.matmul(py[:], ones_t[0:1, 0:128],
                                         bor_sb[0:1, l * 512:(l + 1) * 512],
                                         start=False, stop=True)
                    nc.vector.tensor_copy(yt[:], py[:])
                    nc.sync.dma_start(
                        out=y_d[st * 128:(st + 1) * 128, l * 512:(l + 1) * 512],
                        in_=yt[:])

            def oproj_units(st, l):
                py = paux.tile([128, 512], f32, tag="aux", name="py")
                yt = wpool.tile([128, 512], f32, tag="y", bufs=3, name="yt")
                yield lambda: emit_oproj_half(st, l, 0, py, yt)
                yield lambda: emit_oproj_half(st, l, 1, py, yt)

            # ---- lead-in: kT p=0 all jc, qT p=0 jc=0 (overlaps input DMA) ----
            for jc in range(4):
                for u in qk_group_units(wk_sb, bk_sb, kT_sb, 0, jc, with_bk):
                    u()
            for u in qk_group_units(wq_sb, bq_sb, qT_sb, 0, 0, with_bq):
                u()

            # ---- weaver: ordered filler units with forced prerequisites.
            # Emission order IS program order for the Tile framework, so a
            # consumer must force-emit ("require") its producers first; the
            # paced pop just smooths PE load between attention iterations.
            units = []          # (key, fn)
            emitted = []

            def add_unit(key, fn):
                units.append((key, fn))
                emitted.append(False)

            def require(pred):
                n = 0
                for idx, (key, fn) in enumerate(units):
                    if not emitted[idx] and pred(key):
                        emitted[idx] = True
                        fn()
                        n += 1
                return n

            def pop_units(n):
                got = 0
                for idx, (key, fn) in enumerate(units):
                    if got >= n:
                        break
                    if not emitted[idx]:
                        emitted[idx] = True
                        fn()
                        got += 1

            for st in range(ST):
                for h, u in enumerate(v_group_units(st)):
                    add_unit(("v", st, h), u)
            for jc in range(1, 4):
                for h, u in enumerate(
                        qk_group_units(wq_sb, bq_sb, qT_sb, 0, jc, with_bq)):
                    add_unit(("qT", 0, jc, h), u)
            for p in range(1, OT):
                for jc in range(4):
                    for h, u in enumerate(
                            qk_group_units(wk_sb, bk_sb, kT_sb, p, jc, with_bk)):
                        add_unit(("kT", p, jc, h), u)
                for jc in range(4):
                    for h, u in enumerate(
                            qk_group_units(wq_sb, bq_sb, qT_sb, p, jc, with_bq)):
                        add_unit(("qT", p, jc, h), u)

            # pre-seed the first v tiles while input DMA is still landing
            require(lambda k: k[0] == "v" and k[1] < 4)

            # ---- attention: c outer, p inner; normalize deferred 1 chunk ----
            pending = []

            def flush_normalize():
                # one pending entry covers a whole chunk (both heads):
                # broadcast both 1/denom rows into one psum tile via K=1
                # ones-matmuls, then scale the aoT block with a single mul.
                # When this completes a q-column (p == last), queue that
                # column's o-proj units — only now are all 4 aoT p-blocks
                # final, so units added here can never FIFO-block the PE.
                p0, c0, jb0, rb0 = pending.pop(0)
                bt = paux.tile([128, 512], f32, tag="aux", name="bt")
                nc.tensor.matmul(bt[0:64, :], ones_t[64:65, 0:64],
                                 rb0[64:65, :], start=True, stop=True)
                nc.tensor.matmul(bt[64:128, :], ones_t[32:33, 0:64],
                                 rb0[32:33, :], start=True, stop=True)
                nc.vector.tensor_mul(
                    aoT_sb[:, p0, jb0], aoT_sb[:, p0, jb0], bt[:])
                if p0 == OT - 1:
                    for st in range(c0 * 4, c0 * 4 + 4):
                        for l in range(2):
                            for h, u in enumerate(oproj_units(st, l)):
                                add_unit(("oproj", c0, st, l, h), u)

            for c in range(4):
                for p in range(OT):
                    jb = slice(c * 512, (c + 1) * 512)
                    # qT for this (p, c) must precede the first scores matmul
                    require(lambda k: k[0] == "qT" and k[1] == p and k[2] == c)
                    if c == 0 and p > 0:
                        require(lambda k: k[0] == "kT" and k[1] == p
                                and k[2] == 0)
                    otA = popool.tile([65, 512], f32, tag="po", name="otA")
                    otB = popool.tile([65, 512], f32, tag="po", name="otB")
                    for i in range(KT):
                        got = 0
                        if c == 0 and p == 0:
                            got += require(
                                lambda k: k[0] == "v" and k[1] <= min(i + 1, 15))
                        if c == 0 and p > 0:
                            got += require(
                                lambda k: k[0] == "kT" and k[1] == p
                                and k[2] == min(3, (i + 3) // 4))
                        if i == 8 and pending:
                            # mid-chunk: previous chunk's reciprocal chain has
                            # settled; emitting the normalize here keeps its
                            # matmuls from FIFO-blocking the PE queue.
                            flush_normalize()
                        pop_units(1 - min(got, 1))
                        stt = pspool.tile([128, 1024], f32, tag="ps",
                                          name="stt")
                        for off in (0, 64):
                            nc.tensor.matmul(
                                stt[:, off * 8:off * 8 + 512],
                                kT_sb[off:off + 64, p, i * 128:(i + 1) * 128],
                                qT_sb[off:off + 64, p, jb],
                                start=True, stop=True,
                            )
                        et = wpool.tile([128, 1024], bf16, tag="exp", bufs=4,
                                        name="et")
                        nc.scalar.activation(et[:], stt[:], Exp, scale=0.125)
                        for ot, hh in ((otA, 0), (otB, 1)):
                            nc.tensor.matmul(
                                ot[:],
                                v_sb[:, i, 2 * p + hh, :],
                                et[:, hh * 512:(hh + 1) * 512],
                                start=(i == 0), stop=(i == KT - 1),
                            )
                    # drain attn rows; gather both denominator rows into the
                    # persistent den tile (rows 65-95 stay 1.0 filler) and
                    # batch-reciprocal them.
                    nc.vector.tensor_copy(aoT_sb[0:64, p, jb], otA[0:64, :])
                    nc.vector.tensor_copy(aoT_sb[64:128, p, jb], otB[0:64, :])
                    nc.vector.tensor_copy(den_sb[64:65, :], otA[64:65, :])
                    nc.vector.tensor_copy(den_sb[96:97, :], otB[64:65, :])
                    rf = wpool.tile([97, 512], f32, tag="rf", bufs=2,
                                    name="rf")
                    nc.vector.reciprocal(rf[64:97, :], den_sb[64:97, :])
                    rb = wpool.tile([65, 512], bf16, tag="rb", bufs=3,
                                    name="rb")
                    nc.vector.tensor_copy(rb[64:65, :], rf[64:65, :])
                    nc.vector.tensor_copy(rb[32:33, :], rf[96:97, :])
                    pending.append((p, c, jb, rb))
            # tail: flush the last normalize (queues o-proj c=3), then drain
            while pending:
                flush_normalize()
            pop_units(len(units))

    nc.compile()
    return nc


def get_nc(with_bq=False, with_bk=False, with_bv=False, with_bo=False):
    key = (with_bq, with_bk, with_bv, with_bo)
    if key not in _CACHED_NC:
        _CACHED_NC[key] = _build_nc(*key)
    return _CACHED_NC[key]


def make_in_maps(x, Wq, bq, Wk, bk, Wv, bv, Wo, bo):
    x = np.asarray(x, dtype=np.float32)
    in_maps = []
    for c in range(N_CORES):
        b, hg = c // 2, c % 2
        sl = slice(hg * HG_D, (hg + 1) * HG_D)
        in_maps.append({
            "xt": np.ascontiguousarray(np.asarray(x[b]).T).astype(BF16),
            "wq": np.ascontiguousarray(np.asarray(Wq)[:, sl]).astype(BF16),
            "wk": np.ascontiguousarray(np.asarray(Wk)[:, sl]).astype(BF16),
            "wv": np.ascontiguousarray(np.asarray(Wv)[:, sl]).astype(BF16),
            "wo": np.ascontiguousarray(np.asarray(Wo)[sl, :]).astype(BF16),
            "bqt": np.ascontiguousarray(
                np.asarray(bq, np.float32)[sl].reshape(OT, 128).T),
            "bkt": np.ascontiguousarray(
                np.asarray(bk, np.float32)[sl].reshape(OT, 128).T),
            "bvr": np.asarray(bv, np.float32)[sl].reshape(1, HG_D).astype(BF16),
            "bor": (np.asarray(bo, np.float32) if hg == 0
                    else np.zeros(D, np.float32)).reshape(1, D).astype(BF16),
        })
    return in_maps


def run_cores(in_maps, trace=False, with_bq=False, with_bk=False,
              with_bv=False, with_bo=False):
    try:
        import ntff_shim
        ntff_shim.install()
    except Exception:
        pass
    from concourse.bass_utils import run_bass_kernel_spmd

    nc = get_nc(with_bq, with_bk, with_bv, with_bo)
    return run_bass_kernel_spmd(nc, in_maps, list(range(N_CORES)), trace=trace)


def combine(results):
    y = np.empty((4, S, D), np.float32)
    for b in range(4):
        y[b] = results[2 * b]["y"] + results[2 * b + 1]["y"]
    return y


def kernel(x, Wq, bq, Wk, bk, Wv, bv, Wo, bo):
    in_maps = make_in_maps(x, Wq, bq, Wk, bk, Wv, bv, Wo, bo)
    flags = dict(
        with_bq=bool(np.any(np.asarray(bq))),
        with_bk=bool(np.any(np.asarray(bk))),
        with_bv=bool(np.any(np.asarray(bv))),
        with_bo=bool(np.any(np.asarray(bo))),
    )
    res = run_cores(in_maps, trace=False, **flags)
    return combine(res.results)


# revision 22
# speedup vs baseline: 1.0383x; 1.0383x over previous
"""Multi-head attention (B=4, S=2048, D=1024, H=16, Hd=64) on 8 trn2 cores.

Sharding: core c = (batch b = c // 2, head-group hg = c % 2). Each core
computes attention for 8 heads of one batch and the corresponding slice of
the output projection; host sums the two partial outputs per batch.

v2: single merged pipeline. The attention phase is ScalarE-bound (exp);
all projection / O-proj matmuls are woven between attention iterations as
filler units so the PE never idles while exp runs. ScalarE does ONLY exp;
every PSUM->SBUF move is on the Vector engine; reciprocal uses the fast
approximation.

Per-core layout (matmuls bf16, fp32 PSUM):
  xt    = x[b].T                  [D=1024, S=2048]
  qT/kT = (Wslice.T @ xt)         [128, 4, 2048]  d-major pair-tiles
  v     = xt.T @ Wv_slice         [128, 16, 8, 65]  (65th col = ones -> denom)
  attn chunk (c, p): i-loop over 16 k-tiles:
     scoresT[k, q] pair tile = kT-half.T @ qT-half (row-split concurrency)
     exp (ScalarE, scale=1/8) -> et bf16
     outT[d, q] += v-block.T @ et ; row 64 accumulates denom
  normalize (deferred 1 chunk): 1/denom via reciprocal_approx_fast,
     broadcast with K=1 ones-matmul, DVE multiply
  y = aoT.T-blocks @ Wo_slice   [2048, 1024] fp32 partial, per c-group
"""

from collections import deque

import numpy as np
import ml_dtypes

S = 2048
D = 1024
HG_D = 512          # head dims per core (8 heads x 64)
NH = 8              # heads per core
KT = S // 128       # 16 k-tiles
DT = D // 128       # 8 contraction tiles for QKV
ST = S // 128       # 16 s-tiles
OT = HG_D // 128    # 4 pair-tiles / contraction tiles for O-proj
N_CORES = 8

BF16 = ml_dtypes.bfloat16

_CACHED_NC = {}


def _build_nc(with_bq=False, with_bk=False, with_bv=False, with_bo=False):
    import concourse.bass as bass  # noqa: F401
    import concourse.mybir as mybir
    import concourse.tile as tile
    from concourse import bacc

    f32 = mybir.dt.float32
    bf16 = mybir.dt.bfloat16
    Exp = mybir.ActivationFunctionType.Exp

    nc = bacc.Bacc("TRN2", target_bir_lowering=False, debug=False,
                   num_devices=N_CORES)

    xt_d = nc.dram_tensor("xt", [D, S], bf16, kind="ExternalInput")
    wq_d = nc.dram_tensor("wq", [D, HG_D], bf16, kind="ExternalInput")
    wk_d = nc.dram_tensor("wk", [D, HG_D], bf16, kind="ExternalInput")
    wv_d = nc.dram_tensor("wv", [D, HG_D], bf16, kind="ExternalInput")
    wo_d = nc.dram_tensor("wo", [HG_D, D], bf16, kind="ExternalInput")
    bq_d = nc.dram_tensor("bqt", [128, OT], f32, kind="ExternalInput")
    bk_d = nc.dram_tensor("bkt", [128, OT], f32, kind="ExternalInput")
    bv_d = nc.dram_tensor("bvr", [1, HG_D], bf16, kind="ExternalInput")
    bo_d = nc.dram_tensor("bor", [1, D], bf16, kind="ExternalInput")
    y_d = nc.dram_tensor("y", [S, D], f32, kind="ExternalOutput")

    with tile.TileContext(nc) as tc:
        with (
            tc.tile_pool(name="cpool", bufs=1) as cpool,
            tc.tile_pool(name="wpool", bufs=2) as wpool,
            tc.tile_pool(name="pspool", bufs=2, space="PSUM") as pspool,
            tc.tile_pool(name="popool", bufs=2, space="PSUM") as popool,
            tc.tile_pool(name="paux", bufs=2, space="PSUM") as paux,
        ):
            # ---- persistent SBUF tiles ----
            xt_sb = cpool.tile([128, DT, S], bf16, name="xt_sb")
            wq_sb = cpool.tile([128, DT, HG_D], bf16, name="wq_sb")
            wk_sb = cpool.tile([128, DT, HG_D], bf16, name="wk_sb")
            wv_sb = cpool.tile([128, DT, HG_D], bf16, name="wv_sb")
            wo_sb = cpool.tile([128, OT, D], bf16, name="wo_sb")
            bq_sb = cpool.tile([128, OT], f32, name="bq_sb")
            bk_sb = cpool.tile([128, OT], f32, name="bk_sb")
            bvr_sb = cpool.tile([1, HG_D], bf16, name="bvr_sb")
            bor_sb = cpool.tile([1, D], bf16, name="bor_sb")
            ones_t = cpool.tile([128, 128], bf16, name="ones_t")
            qT_sb = cpool.tile([128, OT, S], bf16, name="qT_sb")
            kT_sb = cpool.tile([128, OT, S], bf16, name="kT_sb")
            # v with a trailing ones column per head: attnv lhsT [128, 65]
            # whose 65th output row accumulates the softmax denominator.
            v_sb = cpool.tile([128, ST, NH, 65], bf16, name="v_sb")
            aoT_sb = cpool.tile([128, OT, S], bf16, name="aoT_sb")
            den_sb = cpool.tile([97, 512], f32, name="den_sb")

            # ---- loads: xt + wk + wq first (lead-in needs them) ----
            for k in range(DT):
                nc.sync.dma_start(out=xt_sb[:, k, :], in_=xt_d[k * 128:(k + 1) * 128, :])
                nc.sync.dma_start(out=wk_sb[:, k, :], in_=wk_d[k * 128:(k + 1) * 128, :])
                nc.sync.dma_start(out=wq_sb[:, k, :], in_=wq_d[k * 128:(k + 1) * 128, :])
            for k in range(DT):
                nc.sync.dma_start(out=wv_sb[:, k, :], in_=wv_d[k * 128:(k + 1) * 128, :])
            for k in range(OT):
                nc.sync.dma_start(out=wo_sb[:, k, :], in_=wo_d[k * 128:(k + 1) * 128, :])
            if with_bq:
                nc.sync.dma_start(out=bq_sb[:], in_=bq_d[:])
            if with_bk:
                nc.sync.dma_start(out=bk_sb[:], in_=bk_d[:])
            if with_bv:
                nc.sync.dma_start(out=bvr_sb[:], in_=bv_d[:])
            if with_bo:
                nc.sync.dma_start(out=bor_sb[:], in_=bo_d[:])
            nc.gpsimd.memset(ones_t[:], 1.0)
            nc.vector.memset(v_sb[:], 1.0)
            nc.vector.memset(den_sb[:], 1.0)

            # ---- projection emitters (as 2-MM units for weaving) ----
            def emit_qk_quarter(w_sb, b_sb, out_sb, p, jc, quarter, pq, withb):
                for k in range(2 * quarter, 2 * quarter + 2):
                    nc.tensor.matmul(
                        pq[:],
                        w_sb[:, k, p * 128:(p + 1) * 128],
                        xt_sb[:, k, jc * 512:(jc + 1) * 512],
                        start=(k == 0), stop=(k == DT - 1),
                    )
                if quarter == 3:
                    dst = out_sb[:, p, jc * 512:(jc + 1) * 512]
                    if withb:
                        nc.scalar.add(dst, pq[:], b_sb[:, p:p + 1])
                    else:
                        nc.vector.tensor_copy(dst, pq[:])

            def qk_group_units(w_sb, b_sb, out_sb, p, jc, withb):
                pq = paux.tile([128, 512], f32, tag="aux", name="pq")
                for quarter in range(4):
                    yield (lambda q=quarter: emit_qk_quarter(
                        w_sb, b_sb, out_sb, p, jc, q, pq, withb))

            def emit_v_quarter(st, quarter, pv):
                for k in range(2 * quarter, 2 * quarter + 2):
                    nc.tensor.matmul(
                        pv[:],
                        xt_sb[:, k, st * 128:(st + 1) * 128],
                        wv_sb[:, k, :],
                        start=(k == 0),
                        stop=(not with_bv and k == DT - 1),
                    )
                if quarter == 3:
                    if with_bv:
                        nc.tensor.matmul(pv[:], ones_t[0:1, 0:128],
                                         bvr_sb[0:1, :], start=False, stop=True)
                    nc.vector.tensor_copy(
                        v_sb[:, st, :, 0:64],
                        pv.rearrange("p (h c) -> p h c", c=64))

            def v_group_units(st):
                pv = paux.tile([128, 512], f32, tag="aux", name="pv")
                for quarter in range(4):
                    yield (lambda q=quarter: emit_v_quarter(st, q, pv))

            def emit_oproj_half(st, l, half, py, yt):
                ks = range(2) if half == 0 else range(2, 4)
                for kt in ks:
                    nc.tensor.matmul(
                        py[:],
                        aoT_sb[:, kt, st * 128:(st + 1) * 128],
                        wo_sb[:, kt, l * 512:(l + 1) * 512],
                        start=(kt == 0),
                        stop=(not with_bo and kt == OT - 1),
                    )
                if half == 1:
                    if with_bo:
                        nc.tensor.matmul(py[:], ones_t[0:1, 0:128],
                                         bor_sb[0:1, l * 512:(l + 1) * 512],
                                         start=False, stop=True)
                    nc.vector.tensor_copy(yt[:], py[:])
                    nc.sync.dma_start(
                        out=y_d[st * 128:(st + 1) * 128, l * 512:(l + 1) * 512],
                        in_=yt[:])

            def oproj_units(st, l):
                py = paux.tile([128, 512], f32, tag="aux", name="py")
                yt = wpool.tile([128, 512], f32, tag="y", bufs=3, name="yt")
                yield lambda: emit_oproj_half(st, l, 0, py, yt)
                yield lambda: emit_oproj_half(st, l, 1, py, yt)

            # ---- lead-in: kT p=0 all jc, qT p=0 jc=0 (overlaps input DMA) ----
            for jc in range(4):
                for u in qk_group_units(wk_sb, bk_sb, kT_sb, 0, jc, with_bk):
                    u()
            for u in qk_group_units(wq_sb, bq_sb, qT_sb, 0, 0, with_bq):
                u()

            # ---- weaver: ordered filler units with forced prerequisites.
            # Emission order IS program order for the Tile framework, so a
            # consumer must force-emit ("require") its producers first; the
            # paced pop just smooths PE load between attention iterations.
            units = []          # (key, fn)
            emitted = []
            emit_count = [0]

            def add_unit(key, fn):
                units.append((key, fn))
                emitted.append(False)

            def require(pred):
                n = 0
                for idx, (key, fn) in enumerate(units):
                    if not emitted[idx] and pred(key):
                        emitted[idx] = True
                        fn()
                        n += 1
                emit_count[0] += n
                return n

            def pop_units(n):
                got = 0
                for idx, (key, fn) in enumerate(units):
                    if got >= n:
                        break
                    if not emitted[idx]:
                        emitted[idx] = True
                        fn()
                        got += 1
                emit_count[0] += got

            for st in range(ST):
                for h, u in enumerate(v_group_units(st)):
                    add_unit(("v", st, h), u)
            for jc in range(1, 4):
                for h, u in enumerate(
                        qk_group_units(wq_sb, bq_sb, qT_sb, 0, jc, with_bq)):
                    add_unit(("qT", 0, jc, h), u)
            for p in range(1, OT):
                for jc in range(4):
                    for h, u in enumerate(
                            qk_group_units(wk_sb, bk_sb, kT_sb, p, jc, with_bk)):
                        add_unit(("kT", p, jc, h), u)
                for jc in range(4):
                    for h, u in enumerate(
                            qk_group_units(wq_sb, bq_sb, qT_sb, p, jc, with_bq)):
                        add_unit(("qT", p, jc, h), u)

            # pre-seed the first v tiles while input DMA is still landing
            require(lambda k: k[0] == "v" and k[1] < 4)

            # ---- attention: c outer, p inner ----
            # Normalize is deferred one chunk, split in two stages so its
            # long DVE reciprocal never sits in front of proj-unit copies:
            # chunk epilogue does only the drains + tiny den-row gathers;
            # the reciprocal+casts run at i==5 of the NEXT chunk and the
            # normalize matmuls+mul at i==8.
            pending = []

            def emit_recip(ent):
                rf = wpool.tile([97, 512], f32, tag="rf", bufs=2, name="rf")
                nc.vector.reciprocal(rf[64:97, :], den_sb[64:97, :])
                rb = wpool.tile([65, 512], bf16, tag="rb", bufs=3, name="rb")
                nc.vector.tensor_copy(rb[64:65, :], rf[64:65, :])
                nc.vector.tensor_copy(rb[32:33, :], rf[96:97, :])
                ent["rb"] = rb

            def flush_normalize():
                # broadcast both 1/denom rows into one psum tile via K=1
                # ones-matmuls, then scale the aoT block with a single mul.
                # When this completes a q-column (p == last), queue that
                # column's o-proj units — only now are all 4 aoT p-blocks
                # final, so units added here can never FIFO-block the PE.
                ent = pending.pop(0)
                if "rb" not in ent:
                    emit_recip(ent)
                p0, c0, jb0, rb0 = ent["p"], ent["c"], ent["jb"], ent["rb"]
                bt = paux.tile([128, 512], f32, tag="aux", name="bt")
                nc.tensor.matmul(bt[0:64, :], ones_t[64:65, 0:64],
                                 rb0[64:65, :], start=True, stop=True)
                nc.tensor.matmul(bt[64:128, :], ones_t[32:33, 0:64],
                                 rb0[32:33, :], start=True, stop=True)
                nc.vector.tensor_mul(
                    aoT_sb[:, p0, jb0], aoT_sb[:, p0, jb0], bt[:])
                if p0 == OT - 1:
                    for st in range(c0 * 4, c0 * 4 + 4):
                        for l in range(2):
                            for h, u in enumerate(oproj_units(st, l)):
                                add_unit(("oproj", c0, st, l, h), u)

            chunks = [(c, p) for c in range(4) for p in range(OT)]
            for t, (c, p) in enumerate(chunks):
                jb = slice(c * 512, (c + 1) * 512)
                # safety: producers for this chunk (normally pre-emitted)
                require(lambda k: k[0] == "qT" and k[1] == p and k[2] == c)
                if c == 0:
                    require(lambda k: k[0] == "kT" and k[1] == p
                            and k[2] == 0)
                otA = popool.tile([65, 512], f32, tag="po", name="otA")
                otB = popool.tile([65, 512], f32, tag="po", name="otB")
                for i in range(KT):
                    got = 0
                    if c == 0 and p == 0:
                        got += require(
                            lambda k: k[0] == "v" and k[1] <= min(i + 1, 15))
                    if c == 0 and p < OT - 1:
                        # feed the NEXT chunk's kT a jc-group ahead of use
                        if p == 0:
                            if i >= 8:
                                got += require(
                                    lambda k: k[0] == "kT" and k[1] == 1
                                    and k[2] <= min(3, (i - 8) // 2))
                        else:
                            got += require(
                                lambda k: k[0] == "kT" and k[1] == p + 1
                                and k[2] <= min(3, i // 4))
                    if i == 5 and pending and "rb" not in pending[0]:
                        emit_recip(pending[0])
                    if i == 8 and pending:
                        flush_normalize()
                    if i == 12 and t + 1 < len(chunks):
                        cn, pn = chunks[t + 1]
                        require(lambda k: k[0] == "qT" and k[1] == pn
                                and k[2] == cn)
                    if not got:
                        allowed = (t * KT + i + 1) * 236 // 256 + 6
                        if emit_count[0] < allowed:
                            pop_units(1)
                    stt = pspool.tile([128, 1024], f32, tag="ps",
                                      name="stt")
                    for off in (0, 64):
                        nc.tensor.matmul(
                            stt[:, off * 8:off * 8 + 512],
                            kT_sb[off:off + 64, p, i * 128:(i + 1) * 128],
                            qT_sb[off:off + 64, p, jb],
                            start=True, stop=True,
                        )
                    et = wpool.tile([128, 1024], bf16, tag="exp", bufs=4,
                                    name="et")
                    nc.scalar.activation(et[:], stt[:], Exp, scale=0.125)
                    for ot, hh in ((otA, 0), (otB, 1)):
                        nc.tensor.matmul(
                            ot[:],
                            v_sb[:, i, 2 * p + hh, :],
                            et[:, hh * 512:(hh + 1) * 512],
                            start=(i == 0), stop=(i == KT - 1),
                        )
                # epilogue: drain attn rows (frees ot for the next chunk)
                # and gather the two denominator rows; reciprocal deferred.
                nc.vector.tensor_copy(aoT_sb[0:64, p, jb], otA[0:64, :])
                nc.vector.tensor_copy(aoT_sb[64:128, p, jb], otB[0:64, :])
                nc.vector.tensor_copy(den_sb[64:65, :], otA[64:65, :])
                nc.vector.tensor_copy(den_sb[96:97, :], otB[64:65, :])
                pending.append({"p": p, "c": c, "jb": jb})
            # tail: flush the last normalize (queues o-proj c=3), then drain
            while pending:
                flush_normalize()
            pop_units(len(units))

    nc.compile()
    return nc


def get_nc(with_bq=False, with_bk=False, with_bv=False, with_bo=False):
    key = (with_bq, with_bk, with_bv, with_bo)
    if key not in _CACHED_NC:
        _CACHED_NC[key] = _build_nc(*key)
    return _CACHED_NC[key]


def make_in_maps(x, Wq, bq, Wk, bk, Wv, bv, Wo, bo):
    x = np.asarray(x, dtype=np.float32)
    in_maps = []
    for c in range(N_CORES):
        b, hg = c // 2, c % 2
        sl = slice(hg * HG_D, (hg + 1) * HG_D)
        in_maps.append({
            "xt": np.ascontiguousarray(np.asarray(x[b]).T).astype(BF16),
            "wq": np.ascontiguousarray(np.asarray(Wq)[:, sl]).astype(BF16),
            "wk": np.ascontiguousarray(np.asarray(Wk)[:, sl]).astype(BF16),
            "wv": np.ascontiguousarray(np.asarray(Wv)[:, sl]).astype(BF16),
            "wo": np.ascontiguousarray(np.asarray(Wo)[sl, :]).astype(BF16),
            "bqt": np.ascontiguousarray(
                np.asarray(bq, np.float32)[sl].reshape(OT, 128).T),
            "bkt": np.ascontiguousarray(
                np.asarray(bk, np.float32)[sl].reshape(OT, 128).T),
            "bvr": np.asarray(bv, np.float32)[sl].reshape(1, HG_D).astype(BF16),
            "bor": (np.asarray(bo, np.float32) if hg == 0
                    else np.zeros(D, np.float32)).reshape(1, D).astype(BF16),
        })
    return in_maps


def run_cores(in_maps, trace=False, with_bq=False, with_bk=False,
              with_bv=False, with_bo=False):
    try:
        import ntff_shim
        ntff_shim.install()
    except Exception:
        pass
    from concourse.bass_utils import run_bass_kernel_spmd

    nc = get_nc(with_bq, with_bk, with_bv, with_bo)
    return run_bass_kernel_spmd(nc, in_maps, list(range(N_CORES)), trace=trace)


def combine(results):
    y = np.empty((4, S, D), np.float32)
    for b in range(4):
        y[b] = results[2 * b]["y"] + results[2 * b + 1]["y"]
    return y


def kernel(x, Wq, bq, Wk, bk, Wv, bv, Wo, bo):
    in_maps = make_in_maps(x, Wq, bq, Wk, bk, Wv, bv, Wo, bo)
    flags = dict(
        with_bq=bool(np.any(np.asarray(bq))),
        with_bk=bool(np.any(np.asarray(bk))),
        with_bv=bool(np.any(np.asarray(bv))),
        with_bo=bool(np.any(np.asarray(bo))),
    )
    res = run_cores(in_maps, trace=False, **flags)
    return combine(res.results)
